# revision 25
# baseline (speedup 1.0000x reference)
"""DiagonalBiLSTM Trainium2 kernel, v2.

Full inputs in, full output out. Sharding: direction-parallel x row-parallel.
Cores 0-3 run the left-to-right diagonal scan over row blocks of 256
(4 batches each); cores 4-7 run the right-to-left scan (x W-flipped).
Each core computes 256 own rows plus a shrinking ghost region (126-d extra
rows at diagonal step d) so no inter-core communication is needed.

Per diagonal step d (n = 382 - d active columns, 3 column chunks <=128):
    u  = xs[d] + k0 @ h + k1 @ h(+1)        (xs = wm@x + b_i2s + b_s2s,
                                             precomputed on host, fp16)
    g  = w_ih @ u + bias                    (bias via K-hot bias matmuls)
    c  = sig(g_f) * c + sig(g_i) * tanh(g_g)
    h  = sig(g_o) * tanh(c)

All matmul operands fp16 (full-rate at any free dim, LDWEIGHTS hidden);
c state fp32. Column chunks are software-pipelined so the PE never idles
(avoids HAM clock-gate re-throttling) and Act/DVE run concurrently.
"""

import numpy as np

B, CIN, COUT, H, W, DC = 16, 256, 256, 64, 64, 3
WD = 2 * W - 1          # 127 diagonal steps
NCORES = 8
NDIRCORES = 4           # cores per direction
OWN = 256               # own rows per core (4 batches)
GHOST = 126
NDATA = OWN + GHOST     # 382 max active columns
NPAD = 384
NKC = 2                 # channel chunks (256 = 2*128)
NGT = 8                 # gate m-tiles (1024 = 8*128)
NC_MAX = 128            # max chunk width

# gate slot order in PSUM: [i0,i1,f0,f1,o0,o1,g0,g1] (w_ih row blocks)
SLOT_ROWS = [(0, 128), (128, 256), (256, 384), (384, 512),
             (768, 896), (896, 1024), (512, 640), (640, 768)]

_COMPILED = {}


# ----------------------------------------------------------------- host prep

def _i2s_mask_np():
    oc = np.arange(COUT) % DC
    ic = np.arange(CIN) % DC
    return (ic[None, :] <= oc[:, None]).astype(np.float32)


def _wT_tiles(w, nmt):
    # [out=nmt*128, in=256] -> lhsT tile array [k=128, kc=2, mt=nmt, m=128]
    return np.ascontiguousarray(
        w.T.reshape(NKC, 128, nmt, 128).transpose(1, 0, 2, 3))


def _prep_inputs(x, w_i2s, b_i2s, w_ih, b_ih, b_hh, k0, k1, b_s2s):
    x = np.asarray(x, np.float32)
    wm = (np.asarray(w_i2s, np.float32) * _i2s_mask_np())
    bias_u = (np.asarray(b_i2s, np.float32)
              + np.asarray(b_s2s, np.float32))          # [256]
    bias_g = (np.asarray(b_ih, np.float32)
              + np.asarray(b_hh, np.float32))           # [1024]

    k0_t = _wT_tiles(np.asarray(k0, np.float32), 2).astype(np.float16)
    k1_t = _wT_tiles(np.asarray(k1, np.float32), 2).astype(np.float16)

    # gate weight tiles in slot order: wih[k, kc, t, m]
    wih = np.empty((128, NKC, NGT, 128), np.float32)
    for t, (r0, r1) in enumerate(SLOT_ROWS):
        wt = _wT_tiles(np.asarray(w_ih, np.float32)[r0:r1], 1)  # [128,2,1,128]
        wih[:, :, t, :] = wt[:, :, 0, :]
    wih = wih.astype(np.float16)

    # bias stationaries for the 2-matmul selector form: bgw2[k, h, m] =
    # bias_g[slot 4h+k, ch m] for k < 4; sel[k, s, :] = (k == s)
    bgw2 = np.zeros((128, 2, 128), np.float32)
    for t, (r0, r1) in enumerate(SLOT_ROWS):
        bgw2[t % 4, t // 4, :] = bias_g[r0:r1]
    bgw2 = bgw2.astype(np.float16)
    sel = np.zeros((128, 4, NC_MAX), np.float16)
    for s in range(4):
        sel[s, s, :] = 1.0

    # xs = wm @ x + bias_u for both directions: [B, 256, H, W]
    x2 = np.ascontiguousarray(x.transpose(1, 0, 2, 3)).reshape(CIN, -1)
    xs_l = np.ascontiguousarray(
        (wm @ x2).reshape(COUT, B, H, W).transpose(1, 0, 2, 3))
    xs_l += bias_u[None, :, None, None]
    xs_r = xs_l[:, :, :, ::-1]

    in_maps = []
    for core in range(NCORES):
        xs_d = xs_l if core < NDIRCORES else xs_r
        c0 = (core % NDIRCORES) * 4                     # first batch
        # rows r = 0..381 -> (b, h) = ((c0*64*4 + r)//64, r%64); rows beyond
        # B*H are bias-only (nonexistent ghost rows on the last core).
        xd = np.empty((WD, CIN, NPAD), np.float32)
        xd[:] = bias_u[None, :, None]                   # out-of-band fill
        nrows = min(NDATA, B * H - c0 * H)              # 382 or 256
        rb = np.arange(nrows)
        bs, hs = c0 + rb // H, rb % H
        # xd[d, :, r] = xs_d[b, :, h, d - h] when 0 <= d-h < W
        for h in range(H):
            msk = hs == h
            if not msk.any():
                continue
            rows = rb[msk]
            # steps d = h..h+W-1 map to w = 0..W-1
            blk = xs_d[bs[msk], :, h, :]                # [nr, 256, W]
            xd[h:h + W][:, :, rows] = blk.transpose(2, 1, 0)
        in_maps.append({
            "xd": np.ascontiguousarray(xd.reshape(WD, NKC, 128, NPAD)
                                       ).astype(np.float16),
            "k0t": k0_t, "k1t": k1_t, "wih": wih, "bgw2": bgw2,
            "sel": sel,
            "misc": np.full((128, 4),
                            0.0 if core % NDIRCORES == NDIRCORES - 1 else 1.0,
                            np.float32),
        })
    return in_maps


# ----------------------------------------------------------- output assembly

def _assemble(core_outs):
    # core_outs: list of [WD, 2, 128, OWN] -> hs [2dir, WD, 256ch, 1024rows]
    hs = np.zeros((2, WD, CIN, B * H), np.float32)
    for c, o in enumerate(core_outs):
        d = c // NDIRCORES
        j = (c % NDIRCORES) * OWN
        hs[d, :, :, j:j + OWN] = np.asarray(o, np.float32).reshape(
            WD, CIN, OWN)

    def unscramble(hd):             # [WD, 256ch, 1024rows] -> [B, COUT, H, WD]
        a = hd.transpose(0, 2, 1).reshape(WD, B, COUT, H)
        return a.transpose(1, 2, 3, 0)

    def unshift(a):                 # [B, COUT, H, WD] -> [B, COUT, H, W]
        rows = np.arange(H)[:, None]
        cols = rows + np.arange(W)[None, :]
        return a[:, :, rows, cols]

    left = unshift(unscramble(hs[0]))
    right = unshift(unscramble(hs[1]))[:, :, :, ::-1]
    right = np.concatenate(
        [np.zeros_like(right[:, :, :1, :]), right[:, :, :-1, :]], axis=2)
    return left + right


# ------------------------------------------------------- reference-free host
# numpy replica of the device program (fp16 rounding modeled), for debugging

def _core_sim(im, nsteps=WD):
    f16 = np.float16
    xd = im["xd"]                        # [WD, 2, 128, NPAD] fp16

    def unT(t, nmt):                     # [k,kc,mt,m] -> [out, in] f32
        return np.float32(t).transpose(1, 0, 2, 3).reshape(
            CIN, nmt * 128).T

    k0 = unT(im["k0t"], 2)
    k1 = unT(im["k1t"], 2)
    wih_t = np.float32(im["wih"])        # [k, kc, t, m]
    b2 = np.float32(im["bgw2"])          # [k, h, m]
    bg = np.stack([b2[t % 4, t // 4] for t in range(NGT)])
    scale = im["misc"][0, 0]

    def sig(v):
        return 1.0 / (1.0 + np.exp(-v))

    h = np.zeros((CIN, NPAD), np.float32)     # ch x col, col 382+ stays 0
    c = np.zeros((CIN, NPAD), np.float32)
    out = np.zeros((nsteps, CIN, OWN), f16)
    for d in range(nsteps):
        n = NDATA - d
        xs = np.float32(xd[d].reshape(CIN, NPAD))[:, :n]
        u = f16(xs + k0 @ h[:, :n] + k1 @ h[:, 1:n + 1]).astype(np.float32)
        # gates per slot
        g = np.empty((NGT, 128, n), np.float32)
        for t in range(NGT):
            acc = bg[t][:, None] * np.ones((1, n), np.float32)
            for kc in range(NKC):
                acc = acc + wih_t[:, kc, t].T @ u[kc * 128:(kc + 1) * 128]
            g[t] = acc
        gi = np.concatenate([g[0], g[1]])
        gf = np.concatenate([g[2], g[3]])
        go = np.concatenate([g[4], g[5]])
        gg = np.concatenate([g[6], g[7]])
        t1 = f16(f16(sig(gi)) * f16(np.tanh(gg))).astype(np.float32)
        c[:, :n] = c[:, :n] * f16(sig(gf)).astype(np.float32) + t1
        h[:, :n] = f16(f16(sig(go)) * f16(np.tanh(c[:, :n]))).astype(
            np.float32)
        if d < WD - 1:
            h[:, OWN] *= scale
        out[d] = f16(h[:, :OWN])
    return out


# --------------------------------------------------------------- bass kernel

def _build(nsteps=WD):
    import concourse.bacc as bacc
    import concourse.mybir as mybir
    import concourse.tile as tile
    from concourse._compat import get_trn_type

    f32 = mybir.dt.float32
    f16 = mybir.dt.float16
    AF = mybir.ActivationFunctionType

    nc = bacc.Bacc(get_trn_type() or "TRN2", target_bir_lowering=False,
                   debug=False)
    xd = nc.dram_tensor("xd", [WD, NKC, 128, NPAD], f16, kind="ExternalInput")
    k0t = nc.dram_tensor("k0t", [128, NKC, 2, 128], f16, kind="ExternalInput")
    k1t = nc.dram_tensor("k1t", [128, NKC, 2, 128], f16, kind="ExternalInput")
    wih = nc.dram_tensor("wih", [128, NKC, NGT, 128], f16,
                         kind="ExternalInput")
    bgw2 = nc.dram_tensor("bgw2", [128, 2, 128], f16, kind="ExternalInput")
    sel = nc.dram_tensor("sel", [128, 4, NC_MAX], f16, kind="ExternalInput")
    misc = nc.dram_tensor("misc", [128, 4], f32, kind="ExternalInput")
    hs_out = nc.dram_tensor("hs", [WD, NKC, 128, OWN], f16,
                            kind="ExternalOutput")

    with tile.TileContext(nc) as tc:
        with (
            tc.tile_pool(name="wpool", bufs=1) as wpool,
            tc.tile_pool(name="state", bufs=1) as state,
            tc.tile_pool(name="xpool", bufs=4) as xpool,
            tc.tile_pool(name="upool", bufs=4) as upool,
            tc.tile_pool(name="apool", bufs=4) as apool,
            tc.tile_pool(name="upsum", bufs=2, space="PSUM") as upsum,
            tc.tile_pool(name="gpsum", bufs=3, space="PSUM") as gpsum,
        ):
            k0_t = wpool.tile([128, NKC, 2, 128], f16, tag="k0")
            k1_t = wpool.tile([128, NKC, 2, 128], f16, tag="k1")
            wih_t = wpool.tile([128, NKC, NGT, 128], f16, tag="wih")
            bgw2_t = wpool.tile([128, 2, 128], f16, tag="bgw2")
            sel_t = wpool.tile([128, 4, NC_MAX], f16, tag="sel")
            misc_t = wpool.tile([128, 4], f32, tag="misc")
            nc.sync.dma_start(k0_t[:], k0t[:])
            nc.sync.dma_start(k1_t[:], k1t[:])
            nc.sync.dma_start(wih_t[:], wih[:])
            nc.sync.dma_start(bgw2_t[:], bgw2[:])
            nc.sync.dma_start(sel_t[:], sel[:])
            nc.sync.dma_start(misc_t[:], misc[:])

            h = state.tile([128, NKC, NPAD], f16, tag="h")
            cs = state.tile([128, NKC, NPAD], f32, tag="c")
            nc.any.memset(h[:], 0.0)
            nc.any.memset(cs[:], 0.0)

            PF = 2      # xs prefetch depth
            xs_tiles = {}
            for dd in range(min(PF + 1, nsteps)):
                t = xpool.tile([128, NKC, NPAD], f16, tag="xs", name=f"xs_pf")
                for kc in range(NKC):
                    nc.sync.dma_start(t[:, kc], xd[dd, kc])
                xs_tiles[dd] = t

            def emit_umm(uP, lo, hi):
                for m in range(NKC):
                    nc.tensor.matmul(uP[:, m, 0:hi - lo],
                                     k0_t[:, 0, m, :], h[:, 0, lo:hi],
                                     start=True, stop=False)
                    nc.tensor.matmul(uP[:, m, 0:hi - lo],
                                     k0_t[:, 1, m, :], h[:, 1, lo:hi],
                                     start=False, stop=False)
                    nc.tensor.matmul(uP[:, m, 0:hi - lo],
                                     k1_t[:, 0, m, :], h[:, 0, lo + 1:hi + 1],
                                     start=False, stop=False)
                    nc.tensor.matmul(uP[:, m, 0:hi - lo],
                                     k1_t[:, 1, m, :], h[:, 1, lo + 1:hi + 1],
                                     start=False, stop=True)

            def emit_gates(gP, u_sb, n):
                # one bias matmul per PSUM bank: writes 4 slots at once via
                # the k-hot selector rhs; start=True clears the bank.
                for hb in range(2):
                    nc.tensor.matmul(gP[:, 4 * hb:4 * hb + 4, 0:n],
                                     bgw2_t[:, hb, :], sel_t[:, :, 0:n],
                                     start=True, stop=False,
                                     skip_group_check=True)
                for t in range(NGT):
                    nc.tensor.matmul(gP[:, t, 0:n], wih_t[:, 0, t, :],
                                     u_sb[:, 0, 0:n], start=False, stop=False)
                for t in range(NGT):
                    nc.tensor.matmul(gP[:, t, 0:n], wih_t[:, 1, t, :],
                                     u_sb[:, 1, 0:n], start=False, stop=True)

            def emit_uadd(u_sb, uP, xs_t, lo, hi):
                for kc in range(NKC):
                    nc.vector.tensor_add(u_sb[:, kc, 0:hi - lo],
                                         uP[:, kc, 0:hi - lo],
                                         xs_t[:, kc, lo:hi])

            def emit_acts(gP, ifo, gt, n):
                nc.scalar.activation(ifo[:, :, 0:n], gP[:, 0:6, 0:n],
                                     AF.Sigmoid)
                nc.scalar.activation(gt[:, :, 0:n], gP[:, 6:8, 0:n], AF.Tanh)

            def emit_cell1(ifo, gt, t1, lo, hi):
                n = hi - lo
                nc.vector.tensor_mul(t1[:, :, 0:n], ifo[:, 0:2, 0:n],
                                     gt[:, :, 0:n])
                nc.vector.tensor_mul(cs[:, :, lo:hi], cs[:, :, lo:hi],
                                     ifo[:, 2:4, 0:n])
                nc.vector.tensor_add(cs[:, :, lo:hi], cs[:, :, lo:hi],
                                     t1[:, :, 0:n])

            def emit_cell2(ifo, tc_t, lo, hi):
                n = hi - lo
                nc.vector.tensor_mul(h[:, :, lo:hi], ifo[:, 4:6, 0:n],
                                     tc_t[:, :, 0:n])

            for d in range(nsteps):
                n = NDATA - d
                s1, s2 = 126 - d, 254 - d
                chunks = [(0, s1), (s1, s2), (s2, n)]
                chunks = [(lo, hi) for lo, hi in chunks if hi > lo]
                xs_t = xs_tiles.pop(d)
                if d + PF + 1 < nsteps:
                    t = xpool.tile([128, NKC, NPAD], f16, tag="xs", name=f"xs_pf")
                    for kc in range(NKC):
                        nc.sync.dma_start(t[:, kc], xd[d + PF + 1, kc])
                    xs_tiles[d + PF + 1] = t

                uPs, usbs, gPs, ifos, gts = {}, {}, {}, {}, {}

                def chunk_u(i):
                    lo, hi = chunks[i]
                    uPs[i] = upsum.tile([128, NKC, NC_MAX], f32, tag="uP", name=f"uP{d}_{i}")
                    emit_umm(uPs[i], lo, hi)

                def chunk_uadd(i):
                    lo, hi = chunks[i]
                    usbs[i] = upool.tile([128, NKC, NC_MAX], f16, tag="usb", name=f"usb{d}_{i}")
                    emit_uadd(usbs[i], uPs[i], xs_t, lo, hi)

                def chunk_gates(i):
                    lo, hi = chunks[i]
                    gPs[i] = gpsum.tile([128, NGT, NC_MAX], f32, tag="gP", name=f"gP{d}_{i}")
                    emit_gates(gPs[i], usbs[i], hi - lo)

                def chunk_acts(i):
                    lo, hi = chunks[i]
                    ifos[i] = apool.tile([128, 6, NC_MAX], f16, tag="ifo", name=f"ifo{d}_{i}")
                    gts[i] = apool.tile([128, 2, NC_MAX], f16, tag="gt", name=f"gt{d}_{i}")
                    emit_acts(gPs[i], ifos[i], gts[i], hi - lo)

                def chunk_cell1(i):
                    lo, hi = chunks[i]
                    t1 = apool.tile([128, 2, NC_MAX], f16, tag="t1", name=f"t1_{d}_{i}")
                    emit_cell1(ifos[i], gts[i], t1, lo, hi)

                def chunk_tanhc(i):
                    lo, hi = chunks[i]
                    tc_t = apool.tile([128, 2, NC_MAX], f16, tag="tc", name=f"tc{d}_{i}")
                    nc.scalar.activation(tc_t[:, :, 0:hi - lo],
                                         cs[:, :, lo:hi], AF.Tanh)
                    return tc_t

                def chunk_cell2(i, tc_t):
                    lo, hi = chunks[i]
                    emit_cell2(ifos[i], tc_t, lo, hi)

                def acts_bc(i, j0):
                    # sigmoid/tanh for chunk i written into the shared BC
                    # activation tiles at column offset j0
                    lo, hi = chunks[i]
                    n = hi - lo
                    nc.scalar.activation(ifo_bc[:, :, j0:j0 + n],
                                         gPs[i][:, 0:6, 0:n], AF.Sigmoid)
                    nc.scalar.activation(gt_bc[:, :, j0:j0 + n],
                                         gPs[i][:, 6:8, 0:n], AF.Tanh)

                nch = len(chunks)
                if nch == 3:
                    chunk_u(0)
                    chunk_uadd(0)
                    chunk_u(1)
                    chunk_uadd(1)
                    chunk_u(2)
                    chunk_uadd(2)
                    chunk_gates(0)
                    chunk_acts(0)
                    chunk_cell1(0)
                    tc0 = chunk_tanhc(0)
                    chunk_cell2(0, tc0)
                    chunk_gates(1)
                    chunk_acts(1)
                    chunk_cell1(1)
                    tc1 = chunk_tanhc(1)
                    chunk_cell2(1, tc1)
                    chunk_gates(2)
                    chunk_acts(2)
                    chunk_cell1(2)
                    tc2 = chunk_tanhc(2)
                    chunk_cell2(2, tc2)
                else:
                    for i in range(nch):
                        chunk_u(i)
                        chunk_uadd(i)
                        chunk_gates(i)
                        chunk_acts(i)
                        chunk_cell1(i)
                        tci = chunk_tanhc(i)
                        chunk_cell2(i, tci)

                if d < nsteps - 1:
                    nc.gpsimd.tensor_scalar_mul(
                        h[:, :, OWN:OWN + 1], h[:, :, OWN:OWN + 1],
                        misc_t[:, 0:1])

                for kc in range(NKC):
                    nc.gpsimd.dma_start(hs_out[d, kc], h[:, kc, 0:OWN])

    nc.finalize()
    return nc


def _get_compiled(nsteps=WD):
    if nsteps not in _COMPILED:
        _COMPILED[nsteps] = _build(nsteps)
    return _COMPILED[nsteps]


# ------------------------------------------------------------------- driver

def kernel(x, w_i2s, b_i2s, w_ih, b_ih, b_hh, k0, k1, b_s2s):
    from concourse.bass_utils import run_bass_kernel_spmd

    in_maps = _prep_inputs(x, w_i2s, b_i2s, w_ih, b_ih, b_hh, k0, k1, b_s2s)
    nc = _get_compiled()
    res = run_bass_kernel_spmd(nc, in_maps, list(range(NCORES)))
    return _assemble([np.asarray(res.results[c]["hs"]).reshape(WD, CIN, OWN)
                      for c in range(NCORES)])


def kernel_numpy(x, w_i2s, b_i2s, w_ih, b_ih, b_hh, k0, k1, b_s2s):
    """Host-only replica of the device program (debug path)."""
    in_maps = _prep_inputs(x, w_i2s, b_i2s, w_ih, b_ih, b_hh, k0, k1, b_s2s)
    return _assemble([_core_sim(im).reshape(WD, CIN, OWN) for im in in_maps])
